# revision 28
# baseline (speedup 1.0000x reference)
"""MHA kernel for trn2, 8 NeuronCores, head-sharded (2 heads/core).

Per core c (heads 2c, 2c+1):
  qT/kT = (w_{q,k} shard).T @ x.T  -> [128, T] bf16 (rows 0:64 head a, 64:128 head b)
  v     = x @ w_v shard            -> [T, 128]
  per q-tile (512) x key-block (128):
     S^T = kT_blk.T @ qT   (row-tiled pair, K=64 per head, one [128,1024] psum)
     eS: split across two engines:
       - ACT tiles: eS = exp(S^T * ln2/128)  (q pre-scaled by 16*log2(e) on host
         so S^T arrives in units of 1/(128*log2(e)))
       - DVE tiles: eS = bitcast_bf16(int16(S^T + C1)), the Schraudolph exp2
         bit trick: S^T is already in 1/128-of-log2 units, so adding the bf16
         exponent-bias-<<7 constant (16256 - sigma) and rounding to int16
         yields the bf16 bit pattern of ~exp(s)
     y'[65,512] += Vp[kb].T @ eS   (Vp = [V | ones]; row 64 = Z = sum exp)
  at q-tile end: Z -> 1/Z (fast reciprocal) -> broadcast across partitions ->
     yns = y * (1/Z) in bf16 (on-core normalization, heads a+b stacked [128,512])
  out-proj (deferred one q-tile, interleaved): poA/poB = wo_head.T @ yns halves
     (row-tiled concurrent pair); oc = poA + poB (heads summed); DMA bf16.

Host: just sums the per-core outT contributions (all normalization on-core).
"""

import numpy as np
import ml_dtypes

import concourse.bacc as bacc
import concourse.mybir as mybir
from concourse.tile import TileContext
from concourse.bass_utils import run_bass_kernel_spmd

BF16 = ml_dtypes.bfloat16
F32 = mybir.dt.float32
BF = mybir.dt.bfloat16
I16 = mybir.dt.int16
EXP = mybir.ActivationFunctionType.Exp

B, T, C = 1, 4096, 1024
H, D = 16, 64
NCORES = 8
P = 128
CB = C // P          # 8 contraction blocks
KB = T // P          # 32 key blocks
QTS = T // 512       # 8 q tiles

# q is pre-scaled on host so S^T psum = s * 128*log2(e), with s = q.k/sqrt(D)
LOG2E_128 = 128.0 / np.log(2.0)          # 184.6650
ACT_SCALE = float(1.0 / LOG2E_128)       # ln2/128; exp(S^T * this) = exp(s)
SIGMA = 7.0
C1 = float(16256.0 - SIGMA)              # bf16 exponent bias 127<<7, minus sigma
# key blocks handled by the DVE Schraudolph path (kb % 16 in this set)
DVE_SET = frozenset({0, 3, 6, 9, 12})
# out-projection emission slots within the next q-tile's kb loop (delayed so
# the Z-normalization chain of the previous q-tile completes first)
OP_SLOT0, OP_STRIDE = 8, 3

_cached = None


def build_bass():
    global _cached
    if _cached is not None:
        return _cached

    nc = bacc.Bacc("TRN2", target_bir_lowering=False, name="mha_head_sharded")

    xT = nc.dram_tensor("xT", (P, QTS, CB, 512), BF, kind="ExternalInput")
    wq = nc.dram_tensor("wq", (P, CB, P), BF, kind="ExternalInput")
    wk = nc.dram_tensor("wk", (P, CB, P), BF, kind="ExternalInput")
    wv = nc.dram_tensor("wv", (P, CB, P), BF, kind="ExternalInput")
    wo = nc.dram_tensor("wo", (P, C), BF, kind="ExternalInput")
    identD = nc.dram_tensor("ident", (P, P), BF, kind="ExternalInput")
    outT = nc.dram_tensor("outT", (C, T), BF, kind="ExternalOutput")

    with TileContext(nc) as tc:
        with (
            tc.tile_pool(name="const", bufs=1) as const,
            tc.tile_pool(name="work", bufs=3) as work,
            tc.tile_pool(name="psS", bufs=3, space="PSUM") as psS,
            tc.tile_pool(name="psY", bufs=1, space="PSUM") as psY,
        ):
            # ---- load inputs (weights on gpsimd queue, x on sync queue) ----
            wks = const.tile([P, CB, P], BF)
            nc.scalar.dma_start(wks[:], wk[:, :, :])
            wqs = const.tile([P, CB, P], BF)
            nc.scalar.dma_start(wqs[:], wq[:, :, :])
            wvs = const.tile([P, CB, P], BF)
            nc.scalar.dma_start(wvs[:], wv[:, :, :])
            wos = const.tile([P, C], BF)
            nc.scalar.dma_start(wos[:], wo[:, :])
            wosB = const.tile([64, C], BF)
            nc.scalar.dma_start(wosB[:], wo[64:128, :])
            ident = const.tile([P, P], BF)
            nc.scalar.dma_start(ident[:], identD[:, :])
            xTs = const.tile([P, QTS, CB, 512], BF)
            # x chunks: sync {0,3,6}, gpsimd {1,4,7}, scalar {2,5} (after weights)
            xq = [nc.sync, nc.gpsimd, nc.scalar]
            for tt in range(QTS):
                xq[tt % 3].dma_start(xTs[:, tt, :, :], xT[:, tt, :, :])

            # ---- warm up the PE clock (HAM) during the input DMA wait ----
            warm = const.tile([P, 512], BF)
            nc.vector.memset(warm[:], 0.0)
            for _ in range(12):
                pw = psS.tile([P, 1024], F32, tag="s", name="pw")
                nc.tensor.matmul(pw[:, 0:512], warm[:, 0:P], warm[:, :],
                                 start=True, stop=True)

            # ---- kT + qT(first tile) + V, interleaved per tok tile so the
            # PE chases the arriving xT DMA chunks ----
            qTs = const.tile([P, T], BF)
            kTs = const.tile([P, T], BF)
            # ---- S^T + exp emission helper (used by phase pre-lag + main loop) ----
            eS_store = {}

            def emit_s_exp(qt, kb):
                q0 = qt * 512
                k0 = kb * P
                s = psS.tile([P, 1024], F32, tag="s", name="s")
                nc.tensor.matmul(s[:, 0:512], kTs[0:64, k0:k0 + P],
                                 qTs[0:64, q0:q0 + 512], start=True, stop=True)
                nc.tensor.matmul(s[:, 512:1024], kTs[64:128, k0:k0 + P],
                                 qTs[64:128, q0:q0 + 512], start=True, stop=True)
                if (kb % 16) in DVE_SET:
                    eSi = work.tile([P, 1024], I16, tag="es", bufs=17, name="eSi")
                    nc.vector.tensor_scalar(
                        eSi[:], s[:], C1, None, mybir.AluOpType.add)
                    eS_store[(qt, kb)] = (eSi, True)
                else:
                    eS = work.tile([P, 1024], BF, tag="es", bufs=17, name="eS")
                    nc.scalar.activation(eS[:], s[:], EXP, scale=ACT_SCALE)
                    eS_store[(qt, kb)] = (eS, False)

            LAG = 14

            Vp0 = const.tile([P, KB, 65], BF)
            Vp1 = const.tile([P, KB, 65], BF)
            nc.vector.memset(Vp0[:, :, 64:65], 1.0)
            nc.vector.memset(Vp1[:, :, 64:65], 1.0)
            for tt in range(QTS):
                pq = psS.tile([P, 1024], F32, tag="s")
                for cb in range(CB):
                    nc.tensor.matmul(
                        pq[:, 0:512], wks[:, cb, :], xTs[:, tt, cb, :],
                        start=(cb == 0), stop=(cb == CB - 1))
                if tt == 0:
                    for cb in range(CB):
                        nc.tensor.matmul(
                            pq[:, 512:1024], wqs[:, cb, :], xTs[:, 0, cb, :],
                            start=(cb == 0), stop=(cb == CB - 1))
                nc.scalar.copy(kTs[:, tt * 512:(tt + 1) * 512], pq[:, 0:512])
                if tt == 0:
                    nc.vector.tensor_copy(qTs[:, 0:512], pq[:, 512:1024])
                # vT = wv.T @ xT chunk (N=512), then 4 PE transposes -> V
                pvt = psS.tile([P, 1024], F32, tag="s", name="pvt")
                for cb in range(CB):
                    nc.tensor.matmul(
                        pvt[:, 0:512], wvs[:, cb, :], xTs[:, tt, cb, :],
                        start=(cb == 0), stop=(cb == CB - 1))
                vt = const.tile([P, 512], BF, tag="vTs", name="vt", bufs=2)
                nc.vector.tensor_copy(vt[:], pvt[:, 0:512])
                tra = psY.tile([P, 1024], BF, tag="y0", name="tra")
                trb = psY.tile([P, 1024], BF, tag="y1", name="trb")
                for i in range(4):
                    dst = tra if i < 2 else trb
                    nc.tensor.transpose(dst[:, (i % 2) * P:(i % 2 + 1) * P],
                                        vt[:, i * P:(i + 1) * P], ident[:])
                for i in range(4):
                    tb = tt * 4 + i
                    dst = tra if i < 2 else trb
                    nc.vector.tensor_copy(Vp0[:, tb, 0:64],
                                           dst[:, (i % 2) * P:(i % 2) * P + 64])
                    nc.vector.tensor_copy(Vp1[:, tb, 0:64],
                                          dst[:, (i % 2) * P + 64:(i % 2 + 1) * P])


            def emit_qt_proj(tokt):
                # compute qT for tok tile `tokt` using a shared psum slot
                pqd = psS.tile([P, 1024], F32, tag="s", name="pqd")
                for cb in range(CB):
                    nc.tensor.matmul(
                        pqd[:, 0:512], wqs[:, cb, :],
                        xTs[:, tokt, cb, :],
                        start=(cb == 0), stop=(cb == CB - 1))
                nc.vector.tensor_copy(qTs[:, tokt * 512:(tokt + 1) * 512],
                                      pqd[:, 0:512])

            # ---- main loop: attention with deferred out-projection ----
            def emit_outproj(dep, fb, tail=False):
                # heads are pre-normalized by 1/Z, so a single K=128 matmul
                # contracts over both heads' dims and sums them in the PE.
                # The tail flush instead uses two K=64 matmuls (head b read
                # from y1t via relocated weights) to skip the partition-move
                # DMA dependency.
                yns, y1t, q0 = dep
                st = psS.tile([P, 1024], F32, tag="s", name="st")
                po = st[:, 0:512]
                if tail:
                    nc.tensor.matmul(po, wos[0:64, fb * P:(fb + 1) * P],
                                     yns[0:64, :], start=True, stop=False)
                    nc.tensor.matmul(po, wosB[:, fb * P:(fb + 1) * P],
                                     y1t[:, :], start=False, stop=True)
                else:
                    nc.tensor.matmul(po, wos[:, fb * P:(fb + 1) * P],
                                     yns[:, :], start=True, stop=True)
                oc = work.tile([P, 512], BF, tag="oc")
                if tail and fb % 2 == 1:
                    nc.scalar.copy(oc[:], po)
                else:
                    nc.vector.tensor_copy(oc[:], po)
                if fb % 2 == 0:
                    nc.gpsimd.dma_start(outT[fb * P:(fb + 1) * P, q0:q0 + 512], oc[:])
                else:
                    nc.sync.dma_start(outT[fb * P:(fb + 1) * P, q0:q0 + 512], oc[:])

            def attv_rhs(eS, is_i16, lo, hi):
                ap = eS[:, lo:hi]
                return ap.bitcast(BF) if is_i16 else ap

            pending = None
            for qt in range(QTS):
                q0 = qt * 512
                y0 = psY.tile([65, 512], F32, tag="y0")
                y1 = psY.tile([65, 512], F32, tag="y1")
                for kb in range(KB):
                    n = qt * KB + kb
                    if n < LAG:
                        for tgt in (2 * n, 2 * n + 1):
                            emit_s_exp(tgt // KB, tgt % KB)
                    else:
                        tgt = n + LAG
                        if tgt < QTS * KB:
                            emit_s_exp(tgt // KB, tgt % KB)
                    eS, is_i16 = eS_store.pop((qt, kb))
                    nc.tensor.matmul(y0[:, :], Vp0[:, kb, :],
                                     attv_rhs(eS, is_i16, 0, 512),
                                     start=(kb == 0), stop=(kb == KB - 1))
                    nc.tensor.matmul(y1[:, :], Vp1[:, kb, :],
                                     attv_rhs(eS, is_i16, 512, 1024),
                                     start=(kb == 0), stop=(kb == KB - 1))
                    if (pending is not None and kb >= OP_SLOT0
                            and (kb - OP_SLOT0) % OP_STRIDE == 0
                            and (kb - OP_SLOT0) // OP_STRIDE < CB):
                        emit_outproj(pending, (kb - OP_SLOT0) // OP_STRIDE)
                    if kb == 2 and qt + 1 < QTS:
                        emit_qt_proj(qt + 1)

                # ---- drain Y psum to SBUF fp32 immediately (releases the
                # single-buffered psY banks fast), then normalize from SBUF ----
                yraw0 = work.tile([65, 512], F32, tag="yraw0")
                yraw1 = work.tile([65, 512], F32, tag="yraw1")
                nc.scalar.copy(yraw0[:], y0[:, :])
                nc.vector.tensor_copy(yraw1[:], y1[:, :])
                # Z rows to partition 0 (DVE tensor_copy does cross-partition)
                z2 = work.tile([1, 1024], F32, tag="z2")
                nc.vector.tensor_copy(z2[0:1, 0:512], yraw0[64:65, :])
                nc.vector.tensor_copy(z2[0:1, 512:1024], yraw1[64:65, :])
                zr = work.tile([1, 1024], F32, tag="zr")
                nc.vector.reciprocal_approx_fast(zr[:], z2[:])
                zb = work.tile([64, 1024], F32, tag="zb")
                nc.gpsimd.partition_broadcast(zb[:], zr[0:1, :], channels=64)
                yns = work.tile([P, 512], BF, tag="yns", bufs=2)
                y1t = work.tile([64, 512], BF, tag="y1t")
                nc.vector.tensor_tensor(yns[0:64, :], yraw0[0:64, :], zb[:, 0:512],
                                        mybir.AluOpType.mult)
                nc.gpsimd.tensor_tensor(y1t[:], yraw1[0:64, :], zb[:, 512:1024],
                                        mybir.AluOpType.mult)
                nc.gpsimd.dma_start(yns[64:128, :], y1t[:])
                pending = (yns, y1t, q0)

            for fb in range(CB):
                emit_outproj(pending, fb, tail=True)

    nc.compile()
    _cached = nc
    return nc


def make_in_maps(x, w_qkv, w_out):
    """x [1,T,C] f32, w_qkv [C, 3C] f32, w_out [C, C] f32 -> per-core input dicts."""
    x = np.asarray(x, dtype=np.float32)
    w_qkv = np.asarray(w_qkv, dtype=np.float32)
    w_out = np.asarray(w_out, dtype=np.float32)
    # fold attention scale (1/8) and the 128*log2(e) Schraudolph domain into wq
    qscale = np.float32(LOG2E_128 / 8.0)
    # xT pre-arranged to the SBUF layout [P, QTS, CB, 512] so chunk DMAs are
    # contiguous (software DGE on sync/scalar queues chokes on gathers)
    xT = x.reshape(T, C).T.reshape(CB, P, QTS, 512).transpose(1, 2, 0, 3)
    xT = np.ascontiguousarray(xT).astype(BF16)

    def warr(w):  # [C, 128] -> [P, CB, 128]
        return np.ascontiguousarray(
            w.reshape(CB, P, P).transpose(1, 0, 2)).astype(BF16)

    in_maps = []
    for c in range(NCORES):
        cols = slice(P * c, P * (c + 1))
        wq = warr(w_qkv[:, 0:C][:, cols] * qscale)
        wk = warr(w_qkv[:, C:2 * C][:, cols])
        wv = warr(w_qkv[:, 2 * C:3 * C][:, cols])
        wo = np.ascontiguousarray(w_out[P * c:P * (c + 1), :]).astype(BF16)
        in_maps.append({"xT": xT, "wq": wq, "wk": wk, "wv": wv, "wo": wo,
                        "ident": np.eye(P, dtype=BF16)})
    return in_maps


def run(x, w_qkv, w_out, trace=False):
    nc = build_bass()
    in_maps = make_in_maps(x, w_qkv, w_out)
    res = run_bass_kernel_spmd(nc, in_maps, core_ids=list(range(NCORES)), trace=trace)
    acc = np.zeros((C, T), dtype=np.float32)
    for r in res.results:
        acc += r["outT"].astype(np.float32)
    out = np.ascontiguousarray(acc.T).reshape(B, T, C)
    return out, res


def kernel(x, w_qkv, w_out):
    out, _ = run(x, w_qkv, w_out, trace=False)
    return out


# revision 29
# speedup vs baseline: 1.2042x; 1.2042x over previous
"""MHA kernel for trn2, 8 NeuronCores, head-sharded (2 heads/core).

Per core c (heads 2c, 2c+1):
  qT/kT = (w_{q,k} shard).T @ x.T  -> [128, T] bf16 (rows 0:64 head a, 64:128 head b)
  v     = x @ w_v shard            -> [T, 128]
  per q-tile (512) x key-block (128):
     S^T = kT_blk.T @ qT   (row-tiled pair, K=64 per head, one [128,1024] psum)
     eS: split across two engines:
       - ACT tiles: eS = exp(S^T * ln2/128)  (q pre-scaled by 16*log2(e) on host
         so S^T arrives in units of 1/(128*log2(e)))
       - DVE tiles: eS = bitcast_bf16(int16(S^T + C1)), the Schraudolph exp2
         bit trick: S^T is already in 1/128-of-log2 units, so adding the bf16
         exponent-bias-<<7 constant (16256 - sigma) and rounding to int16
         yields the bf16 bit pattern of ~exp(s)
     y'[65,512] += Vp[kb].T @ eS   (Vp = [V | ones]; row 64 = Z = sum exp)
  at q-tile end: Z -> 1/Z (fast reciprocal) -> broadcast across partitions ->
     yns = y * (1/Z) in bf16 (on-core normalization, heads a+b stacked [128,512])
  out-proj (deferred one q-tile, interleaved): poA/poB = wo_head.T @ yns halves
     (row-tiled concurrent pair); oc = poA + poB (heads summed); DMA bf16.

Host: just sums the per-core outT contributions (all normalization on-core).
"""

import numpy as np
import ml_dtypes

import concourse.bacc as bacc
import concourse.mybir as mybir
from concourse.tile import TileContext
from concourse.bass_utils import run_bass_kernel_spmd

BF16 = ml_dtypes.bfloat16
F32 = mybir.dt.float32
BF = mybir.dt.bfloat16
I16 = mybir.dt.int16
EXP = mybir.ActivationFunctionType.Exp

B, T, C = 1, 4096, 1024
H, D = 16, 64
NCORES = 8
P = 128
CB = C // P          # 8 contraction blocks
KB = T // P          # 32 key blocks
QTS = T // 512       # 8 q tiles

# q is pre-scaled on host so S^T psum = s * 128*log2(e), with s = q.k/sqrt(D)
LOG2E_128 = 128.0 / np.log(2.0)          # 184.6650
ACT_SCALE = float(1.0 / LOG2E_128)       # ln2/128; exp(S^T * this) = exp(s)
SIGMA = 7.0
C1 = float(16256.0 - SIGMA)              # bf16 exponent bias 127<<7, minus sigma
# key blocks handled by the DVE Schraudolph path (kb % 16 in this set)
DVE_SET = frozenset({0, 3, 6, 9, 12})
# out-projection emission slots within the next q-tile's kb loop (delayed so
# the Z-normalization chain of the previous q-tile completes first)
OP_SLOT0, OP_STRIDE = 8, 3

_cached = None


def build_bass():
    global _cached
    if _cached is not None:
        return _cached

    nc = bacc.Bacc("TRN2", target_bir_lowering=False, name="mha_head_sharded")

    xT = nc.dram_tensor("xT", (P, QTS, CB, 512), BF, kind="ExternalInput")
    wq = nc.dram_tensor("wq", (P, CB, P), BF, kind="ExternalInput")
    wk = nc.dram_tensor("wk", (P, CB, P), BF, kind="ExternalInput")
    wv = nc.dram_tensor("wv", (P, CB, P), BF, kind="ExternalInput")
    wo = nc.dram_tensor("wo", (P, C), BF, kind="ExternalInput")
    identD = nc.dram_tensor("ident", (P, P), BF, kind="ExternalInput")
    outT = nc.dram_tensor("outT", (C, T), BF, kind="ExternalOutput")

    with TileContext(nc) as tc:
        with (
            tc.tile_pool(name="const", bufs=1) as const,
            tc.tile_pool(name="work", bufs=3) as work,
            tc.tile_pool(name="psS", bufs=3, space="PSUM") as psS,
            tc.tile_pool(name="psY", bufs=1, space="PSUM") as psY,
        ):
            # ---- load inputs (weights on gpsimd queue, x on sync queue) ----
            wks = const.tile([P, CB, P], BF)
            nc.scalar.dma_start(wks[:], wk[:, :, :])
            wqs = const.tile([P, CB, P], BF)
            nc.scalar.dma_start(wqs[:], wq[:, :, :])
            wvs = const.tile([P, CB, P], BF)
            nc.scalar.dma_start(wvs[:], wv[:, :, :])
            wos = const.tile([P, C], BF)
            nc.scalar.dma_start(wos[:], wo[:, :])
            wosB = const.tile([64, C], BF)
            nc.scalar.dma_start(wosB[:], wo[64:128, :])
            ident = const.tile([P, P], BF)
            nc.scalar.dma_start(ident[:], identD[:, :])
            xTs = const.tile([P, QTS, CB, 512], BF)
            # x chunks: sync {0,3,6}, gpsimd {1,4,7}, scalar {2,5} (after weights)
            xq = [nc.sync, nc.gpsimd, nc.scalar]
            for tt in range(QTS):
                xq[tt % 3].dma_start(xTs[:, tt, :, :], xT[:, tt, :, :])

            # ---- warm up the PE clock (HAM) during the input DMA wait ----
            warm = const.tile([P, 512], BF)
            nc.vector.memset(warm[:], 0.0)
            for _ in range(12):
                pw = psS.tile([P, 1024], F32, tag="s", name="pw")
                nc.tensor.matmul(pw[:, 0:512], warm[:, 0:P], warm[:, :],
                                 start=True, stop=True)

            # ---- kT + qT(first tile) + V, interleaved per tok tile so the
            # PE chases the arriving xT DMA chunks ----
            qTs = const.tile([P, T], BF)
            kTs = const.tile([P, T], BF)
            # ---- S^T + exp emission helper (used by phase pre-lag + main loop) ----
            eS_store = {}

            def emit_s_exp(qt, kb):
                q0 = qt * 512
                k0 = kb * P
                s = psS.tile([P, 1024], F32, tag="s", name="s")
                nc.tensor.matmul(s[:, 0:512], kTs[0:64, k0:k0 + P],
                                 qTs[0:64, q0:q0 + 512], start=True, stop=True)
                nc.tensor.matmul(s[:, 512:1024], kTs[64:128, k0:k0 + P],
                                 qTs[64:128, q0:q0 + 512], start=True, stop=True)
                if (kb % 16) in DVE_SET:
                    eSi = work.tile([P, 1024], I16, tag="es", bufs=17, name="eSi")
                    nc.vector.tensor_scalar(
                        eSi[:], s[:], C1, None, mybir.AluOpType.add)
                    eS_store[(qt, kb)] = (eSi, True)
                else:
                    eS = work.tile([P, 1024], BF, tag="es", bufs=17, name="eS")
                    nc.scalar.activation(eS[:], s[:], EXP, scale=ACT_SCALE)
                    eS_store[(qt, kb)] = (eS, False)

            LAG = 14

            Vp0 = const.tile([P, KB, 65], BF)
            Vp1 = const.tile([P, KB, 65], BF)
            nc.vector.memset(Vp0[:, :, 64:65], 1.0)
            nc.vector.memset(Vp1[:, :, 64:65], 1.0)
            for tt in range(QTS):
                pq = psS.tile([P, 1024], F32, tag="s")
                for cb in range(CB):
                    nc.tensor.matmul(
                        pq[:, 0:512], wks[:, cb, :], xTs[:, tt, cb, :],
                        start=(cb == 0), stop=(cb == CB - 1))
                if tt == 0:
                    for cb in range(CB):
                        nc.tensor.matmul(
                            pq[:, 512:1024], wqs[:, cb, :], xTs[:, 0, cb, :],
                            start=(cb == 0), stop=(cb == CB - 1))
                nc.scalar.copy(kTs[:, tt * 512:(tt + 1) * 512], pq[:, 0:512])
                if tt == 0:
                    nc.vector.tensor_copy(qTs[:, 0:512], pq[:, 512:1024])
                # vT = wv.T @ xT chunk (N=512), then 4 PE transposes -> V
                pvt = psS.tile([P, 1024], F32, tag="s", name="pvt")
                for cb in range(CB):
                    nc.tensor.matmul(
                        pvt[:, 0:512], wvs[:, cb, :], xTs[:, tt, cb, :],
                        start=(cb == 0), stop=(cb == CB - 1))
                vt = const.tile([P, 512], BF, tag="vTs", name="vt", bufs=2)
                nc.vector.tensor_copy(vt[:], pvt[:, 0:512])
                tra = psY.tile([P, 1024], BF, tag="y0", name="tra")
                trb = psY.tile([P, 1024], BF, tag="y1", name="trb")
                for i in range(4):
                    dst = tra if i < 2 else trb
                    nc.tensor.transpose(dst[:, (i % 2) * P:(i % 2 + 1) * P],
                                        vt[:, i * P:(i + 1) * P], ident[:])
                for i in range(4):
                    tb = tt * 4 + i
                    dst = tra if i < 2 else trb
                    nc.vector.tensor_copy(Vp0[:, tb, 0:64],
                                           dst[:, (i % 2) * P:(i % 2) * P + 64])
                    nc.vector.tensor_copy(Vp1[:, tb, 0:64],
                                          dst[:, (i % 2) * P + 64:(i % 2 + 1) * P])


            def emit_qt_proj(tokt):
                # compute qT for tok tile `tokt` using a shared psum slot
                pqd = psS.tile([P, 1024], F32, tag="s", name="pqd")
                for cb in range(CB):
                    nc.tensor.matmul(
                        pqd[:, 0:512], wqs[:, cb, :],
                        xTs[:, tokt, cb, :],
                        start=(cb == 0), stop=(cb == CB - 1))
                nc.vector.tensor_copy(qTs[:, tokt * 512:(tokt + 1) * 512],
                                      pqd[:, 0:512])

            # ---- main loop: attention with deferred out-projection ----
            def emit_outproj(dep, fb, tail=False):
                # heads are pre-normalized by 1/Z, so a single K=128 matmul
                # contracts over both heads' dims and sums them in the PE.
                # The tail flush instead uses two K=64 matmuls (head b read
                # from y1t via relocated weights) to skip the partition-move
                # DMA dependency.
                yns, y1t, q0 = dep
                st = psS.tile([P, 1024], F32, tag="s", name="st")
                po = st[:, 0:512]
                if tail:
                    nc.tensor.matmul(po, wos[0:64, fb * P:(fb + 1) * P],
                                     yns[0:64, :], start=True, stop=False)
                    nc.tensor.matmul(po, wosB[:, fb * P:(fb + 1) * P],
                                     y1t[:, :], start=False, stop=True)
                else:
                    nc.tensor.matmul(po, wos[:, fb * P:(fb + 1) * P],
                                     yns[:, :], start=True, stop=True)
                oc = work.tile([P, 512], BF, tag="oc")
                if tail and fb % 2 == 1:
                    nc.scalar.copy(oc[:], po)
                else:
                    nc.vector.tensor_copy(oc[:], po)
                if fb % 2 == 0:
                    nc.gpsimd.dma_start(outT[fb * P:(fb + 1) * P, q0:q0 + 512], oc[:])
                else:
                    nc.sync.dma_start(outT[fb * P:(fb + 1) * P, q0:q0 + 512], oc[:])

            def attv_rhs(eS, is_i16, lo, hi):
                ap = eS[:, lo:hi]
                return ap.bitcast(BF) if is_i16 else ap

            pending = None
            for qt in range(QTS):
                q0 = qt * 512
                y0 = psY.tile([65, 512], F32, tag="y0")
                y1 = psY.tile([65, 512], F32, tag="y1")
                for kb in range(KB):
                    n = qt * KB + kb
                    if n < LAG:
                        for tgt in (2 * n, 2 * n + 1):
                            emit_s_exp(tgt // KB, tgt % KB)
                    else:
                        tgt = n + LAG
                        if tgt < QTS * KB:
                            emit_s_exp(tgt // KB, tgt % KB)
                    eS, is_i16 = eS_store.pop((qt, kb))
                    nc.tensor.matmul(y0[:, :], Vp0[:, kb, :],
                                     attv_rhs(eS, is_i16, 0, 512),
                                     start=(kb == 0), stop=(kb == KB - 1))
                    nc.tensor.matmul(y1[:, :], Vp1[:, kb, :],
                                     attv_rhs(eS, is_i16, 512, 1024),
                                     start=(kb == 0), stop=(kb == KB - 1))
                    if (pending is not None and kb >= OP_SLOT0
                            and (kb - OP_SLOT0) % OP_STRIDE == 0
                            and (kb - OP_SLOT0) // OP_STRIDE < CB):
                        emit_outproj(pending, (kb - OP_SLOT0) // OP_STRIDE)
                    if kb == 2 and qt + 1 < QTS:
                        emit_qt_proj(qt + 1)

                # ---- drain Y psum to SBUF fp32 immediately (releases the
                # single-buffered psY banks fast), then normalize from SBUF ----
                yraw0 = work.tile([65, 512], F32, tag="yraw0")
                yraw1 = work.tile([65, 512], F32, tag="yraw1")
                nc.scalar.copy(yraw0[:], y0[:, :])
                nc.vector.tensor_copy(yraw1[:], y1[:, :])
                # Z rows to partition 0 (DVE tensor_copy does cross-partition)
                z2 = work.tile([1, 1024], F32, tag="z2")
                nc.vector.tensor_copy(z2[0:1, 0:512], yraw0[64:65, :])
                nc.vector.tensor_copy(z2[0:1, 512:1024], yraw1[64:65, :])
                zr = work.tile([1, 1024], F32, tag="zr")
                nc.vector.reciprocal_approx_fast(zr[:], z2[:])
                zb = work.tile([64, 1024], F32, tag="zb")
                nc.gpsimd.partition_broadcast(zb[:], zr[0:1, :], channels=64)
                yns = work.tile([P, 512], BF, tag="yns", bufs=2)
                y1t = work.tile([64, 512], BF, tag="y1t")
                nc.vector.tensor_tensor(yns[0:64, :], yraw0[0:64, :], zb[:, 0:512],
                                        mybir.AluOpType.mult)
                nc.vector.tensor_tensor(y1t[:], yraw1[0:64, :], zb[:, 512:1024],
                                        mybir.AluOpType.mult)
                nc.gpsimd.dma_start(yns[64:128, :], y1t[:])
                pending = (yns, y1t, q0)

            for fb in range(CB):
                emit_outproj(pending, fb, tail=True)

    nc.compile()
    _cached = nc
    return nc


def make_in_maps(x, w_qkv, w_out):
    """x [1,T,C] f32, w_qkv [C, 3C] f32, w_out [C, C] f32 -> per-core input dicts."""
    x = np.asarray(x, dtype=np.float32)
    w_qkv = np.asarray(w_qkv, dtype=np.float32)
    w_out = np.asarray(w_out, dtype=np.float32)
    # fold attention scale (1/8) and the 128*log2(e) Schraudolph domain into wq
    qscale = np.float32(LOG2E_128 / 8.0)
    # xT pre-arranged to the SBUF layout [P, QTS, CB, 512] so chunk DMAs are
    # contiguous (software DGE on sync/scalar queues chokes on gathers)
    xT = x.reshape(T, C).T.reshape(CB, P, QTS, 512).transpose(1, 2, 0, 3)
    xT = np.ascontiguousarray(xT).astype(BF16)

    def warr(w):  # [C, 128] -> [P, CB, 128]
        return np.ascontiguousarray(
            w.reshape(CB, P, P).transpose(1, 0, 2)).astype(BF16)

    in_maps = []
    for c in range(NCORES):
        cols = slice(P * c, P * (c + 1))
        wq = warr(w_qkv[:, 0:C][:, cols] * qscale)
        wk = warr(w_qkv[:, C:2 * C][:, cols])
        wv = warr(w_qkv[:, 2 * C:3 * C][:, cols])
        wo = np.ascontiguousarray(w_out[P * c:P * (c + 1), :]).astype(BF16)
        in_maps.append({"xT": xT, "wq": wq, "wk": wk, "wv": wv, "wo": wo,
                        "ident": np.eye(P, dtype=BF16)})
    return in_maps


def run(x, w_qkv, w_out, trace=False):
    nc = build_bass()
    in_maps = make_in_maps(x, w_qkv, w_out)
    res = run_bass_kernel_spmd(nc, in_maps, core_ids=list(range(NCORES)), trace=trace)
    acc = np.zeros((C, T), dtype=np.float32)
    for r in res.results:
        acc += r["outT"].astype(np.float32)
    out = np.ascontiguousarray(acc.T).reshape(B, T, C)
    return out, res


def kernel(x, w_qkv, w_out):
    out, _ = run(x, w_qkv, w_out, trace=False)
    return out
